# revision 9
# baseline (speedup 1.0000x reference)
"""Trainium2 Bass kernel for nn_ConvSwitchedVAE (moe_routing).

Data-parallel over batch: 512 samples -> 8 cores x 64 samples.
Per core, per sample:
  switch conv (4x4 s2) as 4 matmuls over parity-compacted blocks,
  linear via PE-transpose + 2 matmuls, gumbel argmax/softmax on-chip,
  conv1 (3x3) as 3 matmuls K=96 (Cin x 3 taps) with shifted partition-stacked
  padded image, branch routing folded into a per-sample per-partition scale
  (one-hot * z) on the dense (all-branch) hidden h, conv2 as 3 matmuls K=96
  over all branches (unselected branches are zeroed by the scale), bias via
  a small matmul against the transposed one-hot*z, final out = x + conv2.
"""

import sys

for p in ("/opt/trn_rl_repo",):
    if p not in sys.path:
        sys.path.insert(0, p)

import numpy as np
from einops import rearrange

import concourse.bass as bass
import concourse.mybir as mybir
import concourse.tile as tile
from concourse import bacc
from concourse.bass_utils import run_bass_kernel_spmd
from concourse.masks import make_identity

F32 = mybir.dt.float32
BF16 = mybir.dt.bfloat16
I32 = mybir.dt.int32
AF = mybir.ActivationFunctionType
OP = mybir.AluOpType
AX = mybir.AxisListType

NB, SM, C, HW = 4, 8, 32, 32
NCORES = 8
B_FULL = 512


def build_program(n):
    """Build the per-core Bass program for n samples. Returns compiled nc."""
    nc = bacc.Bacc("TRN2", target_bir_lowering=False, debug=False)

    # ---- DRAM I/O ----
    x_d = nc.dram_tensor("x_sh", [n, C, HW, HW], F32, kind="ExternalInput")
    gn_d = nc.dram_tensor("gn_sh", [n, NB], F32, kind="ExternalInput")
    eps_d = nc.dram_tensor("eps_sh", [n, NB], F32, kind="ExternalInput")
    w1s_d = nc.dram_tensor("w1s", [96, 3, 32], BF16, kind="ExternalInput")
    w2s_d = nc.dram_tensor("w2s", [96, 3, 32], BF16, kind="ExternalInput")
    wsw_d = nc.dram_tensor("wsw", [128, 4], F32, kind="ExternalInput")
    wl_d = nc.dram_tensor("wl", [128, 2, 12], F32, kind="ExternalInput")
    b1c_d = nc.dram_tensor("b1c", [32, 1], F32, kind="ExternalInput")
    b2m_d = nc.dram_tensor("b2m", [4, 32], F32, kind="ExternalInput")
    e4_d = nc.dram_tensor("e4", [4, 32], F32, kind="ExternalInput")
    bl_d = nc.dram_tensor("bl2", [1, 12], F32, kind="ExternalInput")
    bsn_d = nc.dram_tensor("bsn", [n, 1], F32, kind="ExternalInput")

    out_d = nc.dram_tensor("out0", [n, C, HW, HW], F32, kind="ExternalOutput")
    ylog_d = nc.dram_tensor("ylog", [n, NB], F32, kind="ExternalOutput")
    idx_d = nc.dram_tensor("idx", [n, 1], I32, kind="ExternalOutput")
    yhard_d = nc.dram_tensor("yhard", [n, NB], F32, kind="ExternalOutput")
    zm_d = nc.dram_tensor("zmsel", [n, 1], F32, kind="ExternalOutput")
    zlv_d = nc.dram_tensor("zlvsel", [n, 1], F32, kind="ExternalOutput")
    zs_d = nc.dram_tensor("zsel", [n, 1], F32, kind="ExternalOutput")

    with tile.TileContext(nc) as tc:
        with tc.tile_pool(name="persist", bufs=1) as pp:
            # weights / consts resident in SBUF
            w1s = pp.tile([96, 3, 32], BF16, tag="w1s")
            w2s = pp.tile([96, 3, 32], BF16, tag="w2s")
            wsw = pp.tile([128, 4], F32, tag="wsw")
            wl = pp.tile([128, 2, 12], F32, tag="wl")
            b1c = pp.tile([32, 1], F32, tag="b1c")
            b2m = pp.tile([4, 32], F32, tag="b2m")
            e4 = pp.tile([4, 32], F32, tag="e4")
            bl = pp.tile([1, 12], F32, tag="bl")
            bsn = pp.tile([n, 1], F32, tag="bsn")
            gn = pp.tile([n, NB], F32, tag="gn")
            eps = pp.tile([n, NB], F32, tag="eps")
            for t, d in ((w1s, w1s_d), (w2s, w2s_d), (wsw, wsw_d), (wl, wl_d),
                         (b1c, b1c_d), (b2m, b2m_d), (e4, e4_d), (bl, bl_d),
                         (bsn, bsn_d), (gn, gn_d), (eps, eps_d)):
                nc.sync.dma_start(t[:], d[:])

            ident = pp.tile([64, 64], F32, tag="ident")
            make_identity(nc, ident[:])
            ones = pp.tile([1, 64], F32, tag="ones")
            nc.gpsimd.memset(ones[:], 1.0)

            s_raw = pp.tile([n, 256], F32, tag="s_raw")
            s_rows = pp.tile([n, 256], F32, tag="s_rows")
            S_sb = pp.tile([128, 2, n], F32, tag="S_sb")
            ctrl = pp.tile([n, 12], F32, tag="ctrl")
            czh = pp.tile([32, n], F32, tag="czh")
            b2z = pp.tile([32, n], F32, tag="b2z")

            # padded shifted image stacks (even/odd manual double buffer);
            # zero borders are written once and only interiors are rewritten
            x4s = [pp.tile([96, 34, 34], BF16, tag=f"x4_{i}", name=f"x4_{i}")
                   for i in range(6)]
            h3s = [pp.tile([96, 34, 34], BF16, tag=f"h3_{i}", name=f"h3_{i}")
                   for i in range(6)]
            xpads = [pp.tile([32, 34, 34], F32, tag=f"xp_{i}", name=f"xp_{i}")
                     for i in range(6)]
            for t in (*x4s, *h3s, *xpads):
                nc.gpsimd.memset(t[:], 0.0)

            # ---------------- pass A: switch conv ----------------
            with tc.tile_pool(name="pa", bufs=6) as pa, \
                 tc.tile_pool(name="pa_ps", bufs=2,
                              space=bass.MemorySpace.PSUM) as pa_ps:
                for b in range(n):
                    xp = xpads[b % 6]
                    xb = pa.tile([32, 32, 32], F32, tag="xa", name=f"xa{b}")
                    nc.sync.dma_start(xb[:], x_d[b])
                    nc.scalar.activation(xp[0:32, 1:33, 1:33], xb[:], AF.Relu)
                    sw = pa.tile([128, 17, 17], F32, tag="sw", name=f"sw{b}")
                    for u in range(2):
                        for v in range(2):
                            eng = nc.vector if u == 0 else nc.gpsimd
                            eng.tensor_copy(
                                sw[32 * (2 * u + v):32 * (2 * u + v) + 32],
                                xp[0:32, u:34:2, v:34:2])
                    psw = pa_ps.tile([1, 256], F32, tag="psw", name=f"psw{b}")
                    for g in range(4):
                        a, bq = g // 2, g % 2
                        nc.tensor.matmul(
                            psw[:], wsw[:, g:g + 1],
                            sw[:, a:a + 16, bq:bq + 16],
                            start=(g == 0), stop=(g == 3))
                    st = pa.tile([1, 256], F32, tag="st", name=f"st{b}")
                    nc.vector.tensor_copy(st[:], psw[:])
                    nc.sync.dma_start(s_raw[b:b + 1, :], st[:])

            # ---------------- pass B: control ----------------
            with tc.tile_pool(name="pb", bufs=1) as pb, \
                 tc.tile_pool(name="pb_ps", bufs=1,
                              space=bass.MemorySpace.PSUM) as pb_ps:
                nc.scalar.activation(s_rows[:], s_raw[:], AF.Relu, bias=bsn[:])
                for k in range(2):
                    pst = pb_ps.tile([128, n], F32, tag="pst", name=f"pst{k}")
                    nc.tensor.transpose(pst[:], s_rows[:, 128 * k:128 * (k + 1)],
                                        ident[0:n, 0:n])
                    nc.vector.tensor_copy(S_sb[:, k, :], pst[:])
                pc = pb_ps.tile([n, 12], F32, tag="pc")
                nc.tensor.matmul(pc[:], S_sb[:, 0, :], wl[:, 0, :],
                                 start=True, stop=False)
                nc.tensor.matmul(pc[:], S_sb[:, 1, :], wl[:, 1, :],
                                 start=False, stop=False)
                nc.tensor.matmul(pc[:], ones[:, 0:n], bl[:],
                                 start=False, stop=True)
                nc.vector.tensor_copy(ctrl[:], pc[:])

                yn = pb.tile([n, NB], F32, tag="yn")
                nc.vector.tensor_add(yn[:], ctrl[:, 0:4], gn[:])
                mx = pb.tile([n, 1], F32, tag="mx")
                nc.vector.tensor_reduce(mx[:], yn[:], AX.X, OP.max)
                oh = pb.tile([n, NB], F32, tag="oh")
                nc.vector.tensor_scalar(oh[:], yn[:], mx[:], None, OP.is_equal)

                ioti = pb.tile([n, NB], I32, tag="ioti")
                nc.gpsimd.iota(ioti[:], pattern=[[1, NB]], base=0,
                               channel_multiplier=0)
                iotf = pb.tile([n, NB], F32, tag="iotf")
                nc.vector.tensor_copy(iotf[:], ioti[:])
                tmp4 = pb.tile([n, NB], F32, tag="tmp4")
                nc.vector.tensor_mul(tmp4[:], oh[:], iotf[:])
                idxf = pb.tile([n, 1], F32, tag="idxf")
                nc.vector.tensor_reduce(idxf[:], tmp4[:], AX.X, OP.add)
                idxi = pb.tile([n, 1], I32, tag="idxi")
                nc.vector.tensor_copy(idxi[:], idxf[:])

                ysub = pb.tile([n, NB], F32, tag="ysub")
                nc.vector.tensor_scalar(ysub[:], yn[:], mx[:], None, OP.subtract)
                ex = pb.tile([n, NB], F32, tag="ex")
                nc.scalar.activation(ex[:], ysub[:], AF.Exp)
                ssum = pb.tile([n, 1], F32, tag="ssum")
                nc.vector.tensor_reduce(ssum[:], ex[:], AX.X, OP.add)
                rcp = pb.tile([n, 1], F32, tag="rcp")
                nc.vector.reciprocal(rcp[:], ssum[:])
                ysoft = pb.tile([n, NB], F32, tag="ysoft")
                nc.vector.tensor_scalar(ysoft[:], ex[:], rcp[:], None, OP.mult)
                yh = pb.tile([n, NB], F32, tag="yh")
                nc.vector.tensor_sub(yh[:], oh[:], ysoft[:])
                nc.vector.tensor_add(yh[:], yh[:], ysoft[:])

                sig = pb.tile([n, NB], F32, tag="sig")
                nc.scalar.activation(sig[:], ctrl[:, 8:12], AF.Exp, scale=0.5)
                z = pb.tile([n, NB], F32, tag="z")
                nc.vector.tensor_mul(z[:], eps[:], sig[:])
                nc.vector.tensor_add(z[:], z[:], ctrl[:, 4:8])
                cz = pb.tile([n, NB], F32, tag="cz")
                nc.vector.tensor_mul(cz[:], oh[:], z[:])

                zsel = pb.tile([n, 1], F32, tag="zsel")
                nc.vector.tensor_reduce(zsel[:], cz[:], AX.X, OP.add)
                t4b = pb.tile([n, NB], F32, tag="t4b")
                nc.vector.tensor_mul(t4b[:], oh[:], ctrl[:, 4:8])
                zmsel = pb.tile([n, 1], F32, tag="zmsel")
                nc.vector.tensor_reduce(zmsel[:], t4b[:], AX.X, OP.add)
                t4c = pb.tile([n, NB], F32, tag="t4c")
                nc.vector.tensor_mul(t4c[:], oh[:], ctrl[:, 8:12])
                zlvsel = pb.tile([n, 1], F32, tag="zlvsel")
                nc.vector.tensor_reduce(zlvsel[:], t4c[:], AX.X, OP.add)

                # one-hot*z transposed -> per-partition scale + bias columns
                p4 = pb_ps.tile([4, n], F32, tag="p4")
                nc.tensor.transpose(p4[:], cz[:], ident[0:n, 0:n])
                czT = pb.tile([4, n], F32, tag="czT")
                nc.vector.tensor_copy(czT[:], p4[:])
                p32a = pb_ps.tile([32, n], F32, tag="p32a")
                nc.tensor.matmul(p32a[:], e4[:], czT[:])
                nc.vector.tensor_copy(czh[:], p32a[:])
                p32b = pb_ps.tile([32, n], F32, tag="p32b")
                nc.tensor.matmul(p32b[:], b2m[:], czT[:])
                nc.vector.tensor_copy(b2z[:], p32b[:])

                nc.sync.dma_start(ylog_d[:], ctrl[:, 0:4])
                nc.sync.dma_start(idx_d[:], idxi[:])
                nc.sync.dma_start(yhard_d[:], yh[:])
                nc.sync.dma_start(zm_d[:], zmsel[:])
                nc.sync.dma_start(zlv_d[:], zlvsel[:])
                nc.sync.dma_start(zs_d[:], zsel[:])

            # ---------------- pass C: branch convs ----------------
            with tc.tile_pool(name="pcl", bufs=8) as pcl, \
                 tc.tile_pool(name="pc_ps", bufs=2,
                              space=bass.MemorySpace.PSUM) as pc_ps:
                for b in range(n):
                    x4 = x4s[b % 6]
                    h3 = h3s[b % 6]
                    xb = pcl.tile([32, 32, 32], F32, tag="xc", name=f"xc{b}")
                    nc.sync.dma_start(xb[:], x_d[b])
                    nc.scalar.activation(x4[0:32, 1:33, 1:33], xb[:], AF.Relu)
                    nc.vector.tensor_copy(x4[32:64, :, 0:33], x4[0:32, :, 1:34])
                    nc.gpsimd.tensor_copy(x4[64:96, :, 0:32], x4[0:32, :, 2:34])

                    ph = pc_ps.tile([32, 32, 32], F32, tag="ph", name=f"ph{b}")
                    for yo in (0, 16):
                        for ky in range(3):
                            nc.tensor.matmul(
                                ph[:, yo:yo + 16, :], w1s[:, ky, :],
                                x4[:, ky + yo:ky + yo + 16, 0:32],
                                start=(ky == 0), stop=(ky == 2))
                    nc.scalar.activation(h3[0:32, 1:33, 1:33], ph[:],
                                         AF.Relu, bias=b1c[:])
                    nc.vector.tensor_scalar(h3[0:32], h3[0:32],
                                            czh[:, b:b + 1], None, OP.mult)
                    nc.vector.tensor_copy(h3[32:64, :, 0:33], h3[0:32, :, 1:34])
                    nc.vector.tensor_copy(h3[64:96, :, 0:32], h3[0:32, :, 2:34])

                    po = pc_ps.tile([32, 32, 32], F32, tag="po", name=f"po{b}")
                    for yo in (0, 16):
                        for ky in range(3):
                            nc.tensor.matmul(
                                po[:, yo:yo + 16, :], w2s[:, ky, :],
                                h3[:, ky + yo:ky + yo + 16, 0:32],
                                start=(ky == 0), stop=(ky == 2))
                    ob = pcl.tile([32, 32, 32], F32, tag="ob", name=f"ob{b}")
                    nc.vector.scalar_tensor_tensor(
                        ob[:], po[:], b2z[:, b:b + 1], xb[:], OP.add, OP.add)
                    nc.sync.dma_start(out_d[b], ob[:])

    nc.compile()
    return nc


def prep_weights(W1, b1, W2, b2, Ws, bs, Wl, bl, n):
    import ml_dtypes
    w1s = np.ascontiguousarray(
        rearrange(W1, "nb sm c ky kx -> (kx c) ky (nb sm)")
        .astype(ml_dtypes.bfloat16))
    w2s = np.ascontiguousarray(
        rearrange(W2, "nb c sm ky kx -> (kx nb sm) ky c")
        .astype(ml_dtypes.bfloat16))
    wsw = np.ascontiguousarray(
        rearrange(Ws[0], "c (a u) (bq v) -> (u v c) (a bq)", u=2, v=2)
        .astype(np.float32))
    wl = np.ascontiguousarray(
        rearrange(Wl, "(k p) m -> p k m", k=2).astype(np.float32))
    b1c = np.ascontiguousarray(b1.reshape(32, 1).astype(np.float32))
    b2m = np.ascontiguousarray(b2.astype(np.float32))
    e4 = np.zeros((4, 32), np.float32)
    for i in range(4):
        e4[i, 8 * i:8 * i + 8] = 1.0
    bl2 = np.ascontiguousarray(bl.reshape(1, 12).astype(np.float32))
    bsn = np.full((n, 1), float(bs[0]), np.float32)
    return dict(w1s=w1s, w2s=w2s, wsw=wsw, wl=wl, b1c=b1c, b2m=b2m, e4=e4,
                bl2=bl2, bsn=bsn)


_CACHE = {}


def _get_program(n):
    if n not in _CACHE:
        _CACHE[n] = build_program(n)
    return _CACHE[n]


def kernel(x, gumbel_noise, eps, W1, b1, W2, b2, Ws, bs, Wl, bl):
    x = np.asarray(x, np.float32)
    gumbel_noise = np.asarray(gumbel_noise, np.float32)
    eps = np.asarray(eps, np.float32)
    B = x.shape[0]
    npc = B // NCORES
    nc = _get_program(npc)
    wmap = prep_weights(np.asarray(W1), np.asarray(b1), np.asarray(W2),
                        np.asarray(b2), np.asarray(Ws), np.asarray(bs),
                        np.asarray(Wl), np.asarray(bl), npc)
    in_maps = []
    for k in range(NCORES):
        sl = slice(k * npc, (k + 1) * npc)
        m = dict(wmap)
        m["x_sh"] = np.ascontiguousarray(x[sl])
        m["gn_sh"] = np.ascontiguousarray(gumbel_noise[sl])
        m["eps_sh"] = np.ascontiguousarray(eps[sl])
        in_maps.append(m)
    res = run_bass_kernel_spmd(nc, in_maps, list(range(NCORES))).results
    out = np.concatenate([r["out0"] for r in res], 0)
    ylog = np.concatenate([r["ylog"] for r in res], 0)
    idx = np.concatenate([r["idx"] for r in res], 0).astype(np.int32)
    yhard = np.concatenate([r["yhard"] for r in res], 0)
    zm = np.concatenate([r["zmsel"] for r in res], 0)
    zlv = np.concatenate([r["zlvsel"] for r in res], 0)
    zs = np.concatenate([r["zsel"] for r in res], 0)
    return out, ylog, idx, yhard, zm, zlv, zs


# revision 10
# speedup vs baseline: 1.0039x; 1.0039x over previous
"""Trainium2 Bass kernel for nn_ConvSwitchedVAE (moe_routing).

Data-parallel over batch: 512 samples -> 8 cores x 64 samples.
Per core, per sample:
  switch conv (4x4 s2) as 4 matmuls over parity-compacted blocks,
  linear via PE-transpose + 2 matmuls, gumbel argmax/softmax on-chip,
  conv1 (3x3) as 3 matmuls K=96 (Cin x 3 taps) with shifted partition-stacked
  padded image, branch routing folded into a per-sample per-partition scale
  (one-hot * z) on the dense (all-branch) hidden h, conv2 as 3 matmuls K=96
  over all branches (unselected branches are zeroed by the scale), bias via
  a small matmul against the transposed one-hot*z, final out = x + conv2.
"""

import sys

for p in ("/opt/trn_rl_repo",):
    if p not in sys.path:
        sys.path.insert(0, p)

import numpy as np
from einops import rearrange

import concourse.bass as bass
import concourse.mybir as mybir
import concourse.tile as tile
from concourse import bacc
from concourse.bass_utils import run_bass_kernel_spmd
from concourse.masks import make_identity

F32 = mybir.dt.float32
BF16 = mybir.dt.bfloat16
I32 = mybir.dt.int32
AF = mybir.ActivationFunctionType
OP = mybir.AluOpType
AX = mybir.AxisListType

NB, SM, C, HW = 4, 8, 32, 32
NCORES = 8
B_FULL = 512


def build_program(n):
    """Build the per-core Bass program for n samples. Returns compiled nc."""
    nc = bacc.Bacc("TRN2", target_bir_lowering=False, debug=False)

    # ---- DRAM I/O ----
    x_d = nc.dram_tensor("x_sh", [n, C, HW, HW], F32, kind="ExternalInput")
    gn_d = nc.dram_tensor("gn_sh", [n, NB], F32, kind="ExternalInput")
    eps_d = nc.dram_tensor("eps_sh", [n, NB], F32, kind="ExternalInput")
    w1s_d = nc.dram_tensor("w1s", [96, 3, 32], BF16, kind="ExternalInput")
    w2s_d = nc.dram_tensor("w2s", [96, 3, 32], BF16, kind="ExternalInput")
    wsw_d = nc.dram_tensor("wsw", [128, 4], F32, kind="ExternalInput")
    wl_d = nc.dram_tensor("wl", [128, 2, 12], F32, kind="ExternalInput")
    b1c_d = nc.dram_tensor("b1c", [32, 1], F32, kind="ExternalInput")
    b2m_d = nc.dram_tensor("b2m", [4, 32], F32, kind="ExternalInput")
    e4_d = nc.dram_tensor("e4", [4, 32], F32, kind="ExternalInput")
    bl_d = nc.dram_tensor("bl2", [1, 12], F32, kind="ExternalInput")
    bsn_d = nc.dram_tensor("bsn", [n, 1], F32, kind="ExternalInput")

    out_d = nc.dram_tensor("out0", [n, C, HW, HW], F32, kind="ExternalOutput")
    ylog_d = nc.dram_tensor("ylog", [n, NB], F32, kind="ExternalOutput")
    idx_d = nc.dram_tensor("idx", [n, 1], I32, kind="ExternalOutput")
    yhard_d = nc.dram_tensor("yhard", [n, NB], F32, kind="ExternalOutput")
    zm_d = nc.dram_tensor("zmsel", [n, 1], F32, kind="ExternalOutput")
    zlv_d = nc.dram_tensor("zlvsel", [n, 1], F32, kind="ExternalOutput")
    zs_d = nc.dram_tensor("zsel", [n, 1], F32, kind="ExternalOutput")

    with tile.TileContext(nc) as tc:
        with tc.tile_pool(name="persist", bufs=1) as pp:
            # weights / consts resident in SBUF
            w1s = pp.tile([96, 3, 32], BF16, tag="w1s")
            w2s = pp.tile([96, 3, 32], BF16, tag="w2s")
            wsw = pp.tile([128, 4], F32, tag="wsw")
            wl = pp.tile([128, 2, 12], F32, tag="wl")
            b1c = pp.tile([32, 1], F32, tag="b1c")
            b2m = pp.tile([4, 32], F32, tag="b2m")
            e4 = pp.tile([4, 32], F32, tag="e4")
            bl = pp.tile([1, 12], F32, tag="bl")
            bsn = pp.tile([n, 1], F32, tag="bsn")
            gn = pp.tile([n, NB], F32, tag="gn")
            eps = pp.tile([n, NB], F32, tag="eps")
            for t, d in ((w1s, w1s_d), (w2s, w2s_d), (wsw, wsw_d), (wl, wl_d),
                         (b1c, b1c_d), (b2m, b2m_d), (e4, e4_d), (bl, bl_d),
                         (bsn, bsn_d), (gn, gn_d), (eps, eps_d)):
                nc.sync.dma_start(t[:], d[:])

            ident = pp.tile([64, 64], F32, tag="ident")
            make_identity(nc, ident[:])
            ones = pp.tile([1, 64], F32, tag="ones")
            nc.gpsimd.memset(ones[:], 1.0)

            s_raw = pp.tile([n, 256], F32, tag="s_raw")
            s_rows = pp.tile([n, 256], F32, tag="s_rows")
            S_sb = pp.tile([128, 2, n], F32, tag="S_sb")
            ctrl = pp.tile([n, 12], F32, tag="ctrl")
            czh = pp.tile([32, n], F32, tag="czh")
            b2z = pp.tile([32, n], F32, tag="b2z")

            # padded shifted image stacks (even/odd manual double buffer);
            # zero borders are written once and only interiors are rewritten
            x4s = [pp.tile([96, 34, 34], BF16, tag=f"x4_{i}", name=f"x4_{i}")
                   for i in range(6)]
            h3s = [pp.tile([96, 34, 34], BF16, tag=f"h3_{i}", name=f"h3_{i}")
                   for i in range(6)]
            xpads = [pp.tile([32, 34, 34], F32, tag=f"xp_{i}", name=f"xp_{i}")
                     for i in range(6)]
            for t in (*x4s, *h3s, *xpads):
                nc.gpsimd.memset(t[:], 0.0)

            # ---------------- pass A: switch conv ----------------
            with tc.tile_pool(name="pa", bufs=6) as pa, \
                 tc.tile_pool(name="pa_ps", bufs=2,
                              space=bass.MemorySpace.PSUM) as pa_ps:
                for b in range(n):
                    xp = xpads[b % 6]
                    xb = pa.tile([32, 32, 32], F32, tag="xa", name=f"xa{b}")
                    nc.sync.dma_start(xb[:], x_d[b])
                    nc.scalar.activation(xp[0:32, 1:33, 1:33], xb[:], AF.Relu)
                    sw = pa.tile([128, 17, 17], F32, tag="sw", name=f"sw{b}")
                    for u in range(2):
                        for v in range(2):
                            eng = nc.vector if u == 0 else nc.gpsimd
                            eng.tensor_copy(
                                sw[32 * (2 * u + v):32 * (2 * u + v) + 32],
                                xp[0:32, u:34:2, v:34:2])
                    psw = pa_ps.tile([1, 256], F32, tag="psw", name=f"psw{b}")
                    for g in range(4):
                        a, bq = g // 2, g % 2
                        nc.tensor.matmul(
                            psw[:], wsw[:, g:g + 1],
                            sw[:, a:a + 16, bq:bq + 16],
                            start=(g == 0), stop=(g == 3))
                    st = pa.tile([1, 256], F32, tag="st", name=f"st{b}")
                    nc.vector.tensor_copy(st[:], psw[:])
                    nc.sync.dma_start(s_raw[b:b + 1, :], st[:])

            # ---------------- pass B: control ----------------
            with tc.tile_pool(name="pb", bufs=1) as pb, \
                 tc.tile_pool(name="pb_ps", bufs=1,
                              space=bass.MemorySpace.PSUM) as pb_ps:
                nc.scalar.activation(s_rows[:], s_raw[:], AF.Relu, bias=bsn[:])
                for k in range(2):
                    pst = pb_ps.tile([128, n], F32, tag="pst", name=f"pst{k}")
                    nc.tensor.transpose(pst[:], s_rows[:, 128 * k:128 * (k + 1)],
                                        ident[0:n, 0:n])
                    nc.vector.tensor_copy(S_sb[:, k, :], pst[:])
                pc = pb_ps.tile([n, 12], F32, tag="pc")
                nc.tensor.matmul(pc[:], S_sb[:, 0, :], wl[:, 0, :],
                                 start=True, stop=False)
                nc.tensor.matmul(pc[:], S_sb[:, 1, :], wl[:, 1, :],
                                 start=False, stop=False)
                nc.tensor.matmul(pc[:], ones[:, 0:n], bl[:],
                                 start=False, stop=True)
                nc.vector.tensor_copy(ctrl[:], pc[:])

                yn = pb.tile([n, NB], F32, tag="yn")
                nc.vector.tensor_add(yn[:], ctrl[:, 0:4], gn[:])
                mx = pb.tile([n, 1], F32, tag="mx")
                nc.vector.tensor_reduce(mx[:], yn[:], AX.X, OP.max)
                oh = pb.tile([n, NB], F32, tag="oh")
                nc.vector.tensor_scalar(oh[:], yn[:], mx[:], None, OP.is_equal)

                ioti = pb.tile([n, NB], I32, tag="ioti")
                nc.gpsimd.iota(ioti[:], pattern=[[1, NB]], base=0,
                               channel_multiplier=0)
                iotf = pb.tile([n, NB], F32, tag="iotf")
                nc.vector.tensor_copy(iotf[:], ioti[:])
                tmp4 = pb.tile([n, NB], F32, tag="tmp4")
                nc.vector.tensor_mul(tmp4[:], oh[:], iotf[:])
                idxf = pb.tile([n, 1], F32, tag="idxf")
                nc.vector.tensor_reduce(idxf[:], tmp4[:], AX.X, OP.add)
                idxi = pb.tile([n, 1], I32, tag="idxi")
                nc.vector.tensor_copy(idxi[:], idxf[:])

                ysub = pb.tile([n, NB], F32, tag="ysub")
                nc.vector.tensor_scalar(ysub[:], yn[:], mx[:], None, OP.subtract)
                ex = pb.tile([n, NB], F32, tag="ex")
                nc.scalar.activation(ex[:], ysub[:], AF.Exp)
                ssum = pb.tile([n, 1], F32, tag="ssum")
                nc.vector.tensor_reduce(ssum[:], ex[:], AX.X, OP.add)
                rcp = pb.tile([n, 1], F32, tag="rcp")
                nc.vector.reciprocal(rcp[:], ssum[:])
                ysoft = pb.tile([n, NB], F32, tag="ysoft")
                nc.vector.tensor_scalar(ysoft[:], ex[:], rcp[:], None, OP.mult)
                yh = pb.tile([n, NB], F32, tag="yh")
                nc.vector.tensor_sub(yh[:], oh[:], ysoft[:])
                nc.vector.tensor_add(yh[:], yh[:], ysoft[:])

                sig = pb.tile([n, NB], F32, tag="sig")
                nc.scalar.activation(sig[:], ctrl[:, 8:12], AF.Exp, scale=0.5)
                z = pb.tile([n, NB], F32, tag="z")
                nc.vector.tensor_mul(z[:], eps[:], sig[:])
                nc.vector.tensor_add(z[:], z[:], ctrl[:, 4:8])
                cz = pb.tile([n, NB], F32, tag="cz")
                nc.vector.tensor_mul(cz[:], oh[:], z[:])

                zsel = pb.tile([n, 1], F32, tag="zsel")
                nc.vector.tensor_reduce(zsel[:], cz[:], AX.X, OP.add)
                t4b = pb.tile([n, NB], F32, tag="t4b")
                nc.vector.tensor_mul(t4b[:], oh[:], ctrl[:, 4:8])
                zmsel = pb.tile([n, 1], F32, tag="zmsel")
                nc.vector.tensor_reduce(zmsel[:], t4b[:], AX.X, OP.add)
                t4c = pb.tile([n, NB], F32, tag="t4c")
                nc.vector.tensor_mul(t4c[:], oh[:], ctrl[:, 8:12])
                zlvsel = pb.tile([n, 1], F32, tag="zlvsel")
                nc.vector.tensor_reduce(zlvsel[:], t4c[:], AX.X, OP.add)

                # one-hot*z transposed -> per-partition scale + bias columns
                p4 = pb_ps.tile([4, n], F32, tag="p4")
                nc.tensor.transpose(p4[:], cz[:], ident[0:n, 0:n])
                czT = pb.tile([4, n], F32, tag="czT")
                nc.vector.tensor_copy(czT[:], p4[:])
                p32a = pb_ps.tile([32, n], F32, tag="p32a")
                nc.tensor.matmul(p32a[:], e4[:], czT[:])
                nc.vector.tensor_copy(czh[:], p32a[:])
                p32b = pb_ps.tile([32, n], F32, tag="p32b")
                nc.tensor.matmul(p32b[:], b2m[:], czT[:])
                nc.vector.tensor_copy(b2z[:], p32b[:])

                nc.sync.dma_start(ylog_d[:], ctrl[:, 0:4])
                nc.sync.dma_start(idx_d[:], idxi[:])
                nc.sync.dma_start(yhard_d[:], yh[:])
                nc.sync.dma_start(zm_d[:], zmsel[:])
                nc.sync.dma_start(zlv_d[:], zlvsel[:])
                nc.sync.dma_start(zs_d[:], zsel[:])

            # ---------------- pass C: branch convs ----------------
            with tc.tile_pool(name="pcl", bufs=8) as pcl, \
                 tc.tile_pool(name="pc_ps", bufs=2,
                              space=bass.MemorySpace.PSUM) as pc_ps:
                def emit_tail(bp, h3p, xbp):
                    po = pc_ps.tile([32, 32, 32], F32, tag="po",
                                    name=f"po{bp}")
                    for yo in (0, 16):
                        for ky in range(3):
                            nc.tensor.matmul(
                                po[:, yo:yo + 16, :], w2s[:, ky, :],
                                h3p[:, ky + yo:ky + yo + 16, 0:32],
                                start=(ky == 0), stop=(ky == 2))
                    ob = pcl.tile([32, 32, 32], F32, tag="ob", name=f"ob{bp}")
                    nc.vector.scalar_tensor_tensor(
                        ob[:], po[:], b2z[:, bp:bp + 1], xbp[:], OP.add, OP.add)
                    nc.sync.dma_start(out_d[bp], ob[:])

                prev = None
                for b in range(n):
                    x4 = x4s[b % 6]
                    h3 = h3s[b % 6]
                    xb = pcl.tile([32, 32, 32], F32, tag="xc", name=f"xc{b}")
                    nc.sync.dma_start(xb[:], x_d[b])
                    nc.scalar.activation(x4[0:32, 1:33, 1:33], xb[:], AF.Relu)
                    nc.vector.tensor_copy(x4[32:64, :, 0:33], x4[0:32, :, 1:34])
                    nc.gpsimd.tensor_copy(x4[64:96, :, 0:32], x4[0:32, :, 2:34])

                    ph = pc_ps.tile([32, 32, 32], F32, tag="ph", name=f"ph{b}")
                    for yo in (0, 16):
                        for ky in range(3):
                            nc.tensor.matmul(
                                ph[:, yo:yo + 16, :], w1s[:, ky, :],
                                x4[:, ky + yo:ky + yo + 16, 0:32],
                                start=(ky == 0), stop=(ky == 2))
                    if prev is not None:
                        emit_tail(*prev)
                    nc.scalar.activation(h3[0:32, 1:33, 1:33], ph[:],
                                         AF.Relu, bias=b1c[:])
                    nc.vector.tensor_scalar(h3[0:32], h3[0:32],
                                            czh[:, b:b + 1], None, OP.mult)
                    nc.vector.tensor_copy(h3[32:64, :, 0:33], h3[0:32, :, 1:34])
                    nc.vector.tensor_copy(h3[64:96, :, 0:32], h3[0:32, :, 2:34])
                    prev = (b, h3, xb)
                emit_tail(*prev)

    nc.compile()
    return nc


def prep_weights(W1, b1, W2, b2, Ws, bs, Wl, bl, n):
    import ml_dtypes
    w1s = np.ascontiguousarray(
        rearrange(W1, "nb sm c ky kx -> (kx c) ky (nb sm)")
        .astype(ml_dtypes.bfloat16))
    w2s = np.ascontiguousarray(
        rearrange(W2, "nb c sm ky kx -> (kx nb sm) ky c")
        .astype(ml_dtypes.bfloat16))
    wsw = np.ascontiguousarray(
        rearrange(Ws[0], "c (a u) (bq v) -> (u v c) (a bq)", u=2, v=2)
        .astype(np.float32))
    wl = np.ascontiguousarray(
        rearrange(Wl, "(k p) m -> p k m", k=2).astype(np.float32))
    b1c = np.ascontiguousarray(b1.reshape(32, 1).astype(np.float32))
    b2m = np.ascontiguousarray(b2.astype(np.float32))
    e4 = np.zeros((4, 32), np.float32)
    for i in range(4):
        e4[i, 8 * i:8 * i + 8] = 1.0
    bl2 = np.ascontiguousarray(bl.reshape(1, 12).astype(np.float32))
    bsn = np.full((n, 1), float(bs[0]), np.float32)
    return dict(w1s=w1s, w2s=w2s, wsw=wsw, wl=wl, b1c=b1c, b2m=b2m, e4=e4,
                bl2=bl2, bsn=bsn)


_CACHE = {}


def _get_program(n):
    if n not in _CACHE:
        _CACHE[n] = build_program(n)
    return _CACHE[n]


def kernel(x, gumbel_noise, eps, W1, b1, W2, b2, Ws, bs, Wl, bl):
    x = np.asarray(x, np.float32)
    gumbel_noise = np.asarray(gumbel_noise, np.float32)
    eps = np.asarray(eps, np.float32)
    B = x.shape[0]
    npc = B // NCORES
    nc = _get_program(npc)
    wmap = prep_weights(np.asarray(W1), np.asarray(b1), np.asarray(W2),
                        np.asarray(b2), np.asarray(Ws), np.asarray(bs),
                        np.asarray(Wl), np.asarray(bl), npc)
    in_maps = []
    for k in range(NCORES):
        sl = slice(k * npc, (k + 1) * npc)
        m = dict(wmap)
        m["x_sh"] = np.ascontiguousarray(x[sl])
        m["gn_sh"] = np.ascontiguousarray(gumbel_noise[sl])
        m["eps_sh"] = np.ascontiguousarray(eps[sl])
        in_maps.append(m)
    res = run_bass_kernel_spmd(nc, in_maps, list(range(NCORES))).results
    out = np.concatenate([r["out0"] for r in res], 0)
    ylog = np.concatenate([r["ylog"] for r in res], 0)
    idx = np.concatenate([r["idx"] for r in res], 0).astype(np.int32)
    yhard = np.concatenate([r["yhard"] for r in res], 0)
    zm = np.concatenate([r["zmsel"] for r in res], 0)
    zlv = np.concatenate([r["zlvsel"] for r in res], 0)
    zs = np.concatenate([r["zsel"] for r in res], 0)
    return out, ylog, idx, yhard, zm, zlv, zs
